# revision 1
# baseline (speedup 1.0000x reference)
"""Two-layer GCN (GCNConv x2) on 8 Trainium2 NeuronCores.

vs v1: (a) one-hot scatter matrices precomputed on host, stored fp8 in DRAM,
cast-loaded to bf16 by SWDGE DMA (frees the DVE, which was the sim-critical
engine); (b) the per-layer feature table is all-gathered in 4 block-aligned
chunks so aggregation of block q overlaps the later chunks' collectives;
(c) stage A emits node-partition outputs directly (no PE transpose).

Self-contained: only numpy/ml_dtypes/concourse imports; all shapes hardcoded.
"""

import numpy as np
import ml_dtypes

N_NODES = 100000
N_FEAT_IN = 256
N_FEAT_H = 128
N_FEAT_OUT = 64
N_EDGES = 1600000
N_CORES = 8
NPC = N_NODES // N_CORES  # 12500 nodes per core
NPAD = 12800  # padded rows per core shard (100 windows of 128)
NWIN = NPAD // 128  # 100
QROWS = NPAD // 4  # 3200 rows per collective chunk (25 windows)
NBLK = 4  # table blocks == collective chunks
BLKROWS = N_CORES * QROWS  # 25600 rows per gathered block (< 32767 int16)
JMAX = 40  # max chunks per gather batch

BF16 = ml_dtypes.bfloat16
FP8 = ml_dtypes.float8_e4m3

_CACHE = {}


# ---------------------------------------------------------------------------
# Host preprocessing
# ---------------------------------------------------------------------------

def _wrap16(idx: np.ndarray) -> np.ndarray:
    """dma_gather index layout: idx i -> partition i%16, col i//16,
    replicated to the 8 groups of 16 partitions. Returns [128, n//16]."""
    n = idx.shape[0]
    arr = idx.reshape(n // 16, 16).T
    return np.ascontiguousarray(np.tile(arr, (8, 1)).astype(np.int16))


def _preprocess(edge_index: np.ndarray):
    src = np.concatenate([edge_index[0], np.arange(N_NODES, dtype=np.int64)])
    dst = np.concatenate([edge_index[1], np.arange(N_NODES, dtype=np.int64)])
    deg = np.bincount(dst, minlength=N_NODES)
    dinv = (1.0 / np.sqrt(deg.astype(np.float64))).astype(np.float32)

    core = (dst // NPC).astype(np.int64)
    loc = dst - core * NPC
    win = loc >> 7
    dl = (loc & 127).astype(np.int64)
    # source row inside its block's table: block q holds, core-major, each
    # core's rows [q*QROWS, (q+1)*QROWS)
    sc = src // NPC
    sl = src - sc * NPC
    blk = sl // QROWS
    bloc = (sc * QROWS + (sl % QROWS)).astype(np.int16)

    ncell = NBLK * NWIN
    cell = (core * NBLK + blk) * NWIN + win  # (dstcore, blk, win)
    counts = np.bincount(cell, minlength=N_CORES * ncell).reshape(
        N_CORES, NBLK, NWIN
    )
    ccap = counts.max(axis=0)  # [NBLK, NWIN]
    chunks_bw = (ccap + 127) // 128  # chunks per (blk, win); may be 0
    cap_flat = (chunks_bw * 128).ravel()
    cell_off = np.concatenate([[0], np.cumsum(cap_flat)])  # per (blk, win)
    tot = int(cell_off[-1])

    # slot position of each edge inside its core's array
    order = np.argsort(cell, kind="stable")
    cell_s = cell[order]
    counts_flat = np.bincount(cell, minlength=N_CORES * ncell)
    run_starts = np.concatenate([[0], np.cumsum(counts_flat)])[:-1]
    rank = np.arange(len(cell_s)) - run_starts[cell_s]
    core_s = cell_s // ncell
    cellbw_s = cell_s % ncell
    pos = cell_off[cellbw_s] + rank

    src_arr = np.zeros((N_CORES, tot), np.int16)
    dst_arr = np.full((N_CORES, tot), -1, np.int64)
    src_arr[core_s, pos] = bloc[order]
    dst_arr[core_s, pos] = dl[order]

    # batches: per block, consecutive non-empty cells grouped to <= JMAX chunks
    batches = []  # {blk, off(chunks), cells: [(win, nchunks)...], nch}
    k = 0
    for b in range(NBLK):
        cur = None
        for w in range(NWIN):
            nc_w = int(chunks_bw[b, w])
            if nc_w == 0:
                continue
            if cur is None or cur["nch"] + nc_w > JMAX:
                cur = {"blk": b, "off": k, "cells": [], "nch": 0}
                batches.append(cur)
            cur["cells"].append((w, nc_w))
            cur["nch"] += nc_w
            k += nc_w
    ktot = k
    assert ktot * 128 == tot

    # per-core gather index dram [128, ktot*8]
    idx_dram = np.zeros((N_CORES, 128, ktot * 8), np.int16)
    for c in range(N_CORES):
        cols = []
        for bt in batches:
            nb = bt["nch"]
            s = bt["off"] * 128
            cols.append(_wrap16(src_arr[c, s : s + nb * 128]))
        idx_dram[c] = np.hstack(cols)

    # one-hot scatter matrices, fp8: oh[c, p, k*128 + d] = 1 iff slot
    # (k*128 + p) of core c has dst-local d. Padding slots (-1) stay zero.
    oh = np.zeros((N_CORES, 128, ktot * 128), np.uint8)
    slots = np.arange(tot)
    p_of = slots % 128
    k_of = slots // 128
    for c in range(N_CORES):
        valid = dst_arr[c] >= 0
        oh[c, p_of[valid], k_of[valid] * 128 + dst_arr[c][valid]] = 0x38  # 1.0
    oh_dram = oh.view(FP8)

    meta = {"batches": batches, "ktot": ktot}
    return meta, dinv, idx_dram, oh_dram


# ---------------------------------------------------------------------------
# Device kernel
# ---------------------------------------------------------------------------

def _build_nc(meta, nstages=7, reps=1):
    """nstages: 1=A, 2=+AG1, 3=+aggregation1, 4=+epilogue1, 5=+AG2, 6=+agg2,
    7=full. Partial builds write acc1 prefix to z."""
    import concourse.bacc as bacc
    import concourse.mybir as mybir
    import concourse.tile as tile
    from concourse.masks import make_identity

    ktot = meta["ktot"]
    batches = meta["batches"]

    nc = bacc.Bacc(
        "TRN2", target_bir_lowering=False, debug=False, num_devices=N_CORES,
        num_swdge_queues=2,
    )
    f32, bf16, i16 = mybir.dt.float32, mybir.dt.bfloat16, mybir.dt.int16
    f8 = mybir.dt.float8e4

    xT = nc.dram_tensor("xT", [N_FEAT_IN, NPAD], bf16, kind="ExternalInput")
    w1 = nc.dram_tensor("w1", [N_FEAT_IN, N_FEAT_H], f32, kind="ExternalInput")
    w2p = nc.dram_tensor("w2p", [N_FEAT_H, 128], f32, kind="ExternalInput")
    b1r = nc.dram_tensor("b1r", [128, N_FEAT_H], f32, kind="ExternalInput")
    b2r = nc.dram_tensor("b2r", [128, N_FEAT_OUT], f32, kind="ExternalInput")
    dinv_wr = nc.dram_tensor("dinv_wr", [128, NWIN], f32, kind="ExternalInput")
    idxs = nc.dram_tensor("idxs", [128, ktot * 8], i16, kind="ExternalInput")
    ohd = nc.dram_tensor("ohd", [128, ktot * 128], f8, kind="ExternalInput")
    z = nc.dram_tensor("z", [NPAD, N_FEAT_OUT], f32, kind="ExternalOutput")

    with tile.TileContext(nc) as tc:
        with (
            tc.tile_pool(name="dram", bufs=1, space="DRAM") as dram,
            tc.tile_pool(name="persist", bufs=1) as pers,
            tc.tile_pool(name="work", bufs=2) as work,
            tc.tile_pool(name="gpool", bufs=3) as gpool,
            tc.tile_pool(name="ohpool", bufs=3) as ohpool,
            tc.tile_pool(name="stagea", bufs=6) as sta,
            tc.tile_pool(name="psum_mm", bufs=2, space="PSUM") as psmm,
            tc.tile_pool(name="psum_tr", bufs=2, space="PSUM") as pstr,
            tc.tile_pool(name="psum_cell", bufs=4, space="PSUM") as pscell,
        ):
            g1_sh = [
                dram.tile([QROWS, N_FEAT_H], bf16, name=f"g1_sh{q}")
                for q in range(NBLK)
            ]
            g2_sh = [
                dram.tile([QROWS, 128], bf16, name=f"g2_sh{q}")
                for q in range(NBLK)
            ]
            g1_blk = [
                nc.dram_tensor(
                    f"g1_blk{q}", [BLKROWS, N_FEAT_H], bf16, addr_space="Shared"
                )
                for q in range(NBLK)
            ]
            g2_blk = [
                nc.dram_tensor(
                    f"g2_blk{q}", [BLKROWS, 128], bf16, addr_space="Shared"
                )
                for q in range(NBLK)
            ]

            # ---- constants ----
            w1a = pers.tile([128, N_FEAT_H], bf16, tag="w1a")
            w1b = pers.tile([128, N_FEAT_H], bf16, tag="w1b")
            w2t = pers.tile([N_FEAT_H, 128], bf16, tag="w2t")
            b1t = pers.tile([128, N_FEAT_H], f32, tag="b1t")
            b2t = pers.tile([128, N_FEAT_OUT], f32, tag="b2t")
            dnv = pers.tile([128, NWIN], f32, tag="dnv")
            idn = pers.tile([128, 128], bf16, tag="idn")
            nc.gpsimd.dma_start(out=w1a[:], in_=w1[0:128, :])
            nc.gpsimd.dma_start(out=w1b[:], in_=w1[128:256, :])
            nc.gpsimd.dma_start(out=w2t[:], in_=w2p[:])
            nc.sync.dma_start(out=b1t[:], in_=b1r[:])
            nc.sync.dma_start(out=b2t[:], in_=b2r[:])
            nc.sync.dma_start(out=dnv[:], in_=dinv_wr[:])
            make_identity(nc, idn[:])

            for _rep in range(reps):
                acc1 = []
                for w in range(NWIN):
                    acc1_t = pers.tile([128, N_FEAT_H], f32, tag=f"acc1_{w}")
                    nc.vector.memset(acc1_t[:], 0.0)
                    acc1.append(acc1_t)
                acc2 = []
                for w in range(NWIN):
                    acc2_t = pers.tile([128, 128], f32, tag=f"acc2_{w}")
                    nc.vector.memset(acc2_t[:], 0.0)
                    acc2.append(acc2_t)

                # ---- stage A: g1 = dinv * (x @ W1), node-partition layout ----
                for w in range(NWIN):
                    r0 = sta.tile([128, 128], bf16, tag="rhs0")
                    r1 = sta.tile([128, 128], bf16, tag="rhs1")
                    nc.sync.dma_start(
                        out=r0[:], in_=xT[0:128, w * 128 : (w + 1) * 128]
                    )
                    nc.sync.dma_start(
                        out=r1[:], in_=xT[128:256, w * 128 : (w + 1) * 128]
                    )
                    ps = psmm.tile([128, N_FEAT_H], f32, space="PSUM", tag="mm")
                    nc.tensor.matmul(
                        out=ps[:], lhsT=r0[:], rhs=w1a[:], start=True, stop=False
                    )
                    nc.tensor.matmul(
                        out=ps[:], lhsT=r1[:], rhs=w1b[:], start=False, stop=True
                    )
                    g1w = sta.tile([128, N_FEAT_H], bf16, tag="g1w")
                    nc.vector.tensor_scalar(
                        out=g1w[:],
                        in0=ps[:],
                        scalar1=dnv[:, w : w + 1],
                        scalar2=None,
                        op0=mybir.AluOpType.mult,
                    )
                    q, wq = w // 25, w % 25
                    nc.sync.dma_start(
                        out=g1_sh[q][wq * 128 : (wq + 1) * 128, :], in_=g1w[:]
                    )

                if nstages >= 2:
                    for q in range(NBLK):
                        nc.gpsimd.collective_compute(
                            "AllGather",
                            mybir.AluOpType.bypass,
                            ins=[g1_sh[q].opt()],
                            outs=[g1_blk[q][:]],
                            replica_groups=[list(range(N_CORES))],
                        )

                # ---- aggregation over edge chunk batches ----
                def aggregate(blk_tabs, acc, nfeat):
                    for bt in batches:
                        b, off, nb = bt["blk"], bt["off"], bt["nch"]
                        it = work.tile([128, nb * 8], i16, tag="idx")
                        nc.sync.dma_start(
                            out=it[:], in_=idxs[:, off * 8 : (off + nb) * 8]
                        )
                        G = gpool.tile([128, nb, 128], bf16, tag="G")
                        nc.gpsimd.dma_gather(
                            out_ap=G[:],
                            in_ap=blk_tabs[b][:],
                            idxs_ap=it[:],
                            num_idxs=nb * 128,
                            num_idxs_reg=nb * 128,
                            elem_size=128,
                            single_packet=False,
                            queue_num=1,
                        )
                        OH = ohpool.tile([128, nb, 128], bf16, tag="OH")
                        nc.gpsimd.dma_start(
                            out=OH[:],
                            in_=ohd[:, off * 128 : (off + nb) * 128],
                        )
                        j = 0
                        for w, ncw in bt["cells"]:
                            ps = pscell.tile(
                                [128, nfeat], f32, space="PSUM", tag="cell"
                            )
                            for kk in range(ncw):
                                nc.tensor.matmul(
                                    out=ps[:],
                                    lhsT=OH[:, j, :],
                                    rhs=G[:, j, :nfeat],
                                    start=(kk == 0),
                                    stop=(kk == ncw - 1),
                                )
                                j += 1
                            sl = acc[w][:]
                            nc.vector.tensor_add(out=sl, in0=sl, in1=ps[:])

                if nstages >= 3:
                    aggregate(g1_blk, acc1, N_FEAT_H)

                # ---- layer-1 epilogue: relu(dinv*acc1 + b1) @ W2 * dinv ----
                for w in range(NWIN if nstages >= 4 else 0):
                    aw = acc1[w][:]
                    t1 = sta.tile([128, N_FEAT_H], f32, tag="ep1")
                    nc.vector.tensor_scalar(
                        out=t1[:], in0=aw, scalar1=dnv[:, w : w + 1],
                        scalar2=None, op0=mybir.AluOpType.mult,
                    )
                    nc.vector.tensor_add(out=t1[:], in0=t1[:], in1=b1t[:])
                    t3 = sta.tile([128, N_FEAT_H], bf16, tag="ep3")
                    nc.scalar.activation(
                        out=t3[:], in_=t1[:],
                        func=mybir.ActivationFunctionType.Relu,
                    )
                    trp = pstr.tile([128, 128], bf16, space="PSUM", tag="tr")
                    nc.tensor.transpose(out=trp[:], in_=t3[:], identity=idn[:])
                    t3T = sta.tile([128, 128], bf16, tag="ep3T")
                    nc.vector.tensor_copy(out=t3T[:], in_=trp[:])
                    g2ps = psmm.tile([128, 128], f32, space="PSUM", tag="mm")
                    nc.tensor.matmul(
                        out=g2ps[:], lhsT=t3T[:], rhs=w2t[:], start=True,
                        stop=True,
                    )
                    g2w = sta.tile([128, 128], bf16, tag="g2w")
                    nc.vector.tensor_scalar(
                        out=g2w[:], in0=g2ps[:], scalar1=dnv[:, w : w + 1],
                        scalar2=None, op0=mybir.AluOpType.mult,
                    )
                    q, wq = w // 25, w % 25
                    nc.sync.dma_start(
                        out=g2_sh[q][wq * 128 : (wq + 1) * 128, :], in_=g2w[:]
                    )

                if nstages >= 5:
                    for q in range(NBLK):
                        nc.gpsimd.collective_compute(
                            "AllGather",
                            mybir.AluOpType.bypass,
                            ins=[g2_sh[q].opt()],
                            outs=[g2_blk[q][:]],
                            replica_groups=[list(range(N_CORES))],
                        )

                if nstages >= 6:
                    aggregate(g2_blk, acc2, 128)

                # ---- layer-2 epilogue: z = dinv*acc2 + b2 ----
                for w in range(NWIN if nstages >= 7 else 0):
                    sl = acc2[w][:, :N_FEAT_OUT]
                    nc.vector.tensor_scalar(
                        out=sl, in0=sl, scalar1=dnv[:, w : w + 1],
                        scalar2=None, op0=mybir.AluOpType.mult,
                    )
                    nc.vector.tensor_add(out=sl, in0=sl, in1=b2t[:])
                    nc.sync.dma_start(out=z[w * 128 : (w + 1) * 128, :], in_=sl)
                if nstages < 7:
                    for w in range(NWIN):
                        nc.sync.dma_start(
                            out=z[w * 128 : (w + 1) * 128, :],
                            in_=acc1[w][:, :N_FEAT_OUT],
                        )
    nc.compile()
    return nc


# ---------------------------------------------------------------------------
# Entry point
# ---------------------------------------------------------------------------

def prepare(x, edge_index, W1, b1, W2, b2):
    """Preprocess + build + compile; returns (nc, in_maps)."""
    x = np.asarray(x)
    edge_index = np.asarray(edge_index)
    W1 = np.asarray(W1, dtype=np.float32)
    b1 = np.asarray(b1, dtype=np.float32)
    W2 = np.asarray(W2, dtype=np.float32)
    b2 = np.asarray(b2, dtype=np.float32)

    key = hash(edge_index.tobytes())
    if key not in _CACHE:
        meta, dinv, idx_dram, oh_dram = _preprocess(edge_index)
        nc = _build_nc(meta)
        _CACHE[key] = (meta, dinv, idx_dram, oh_dram, nc)
    meta, dinv, idx_dram, oh_dram, nc = _CACHE[key]

    w2pad = np.zeros((N_FEAT_H, 128), np.float32)
    w2pad[:, :N_FEAT_OUT] = W2
    b1rep = np.tile(b1[None, :], (128, 1)).astype(np.float32)
    b2rep = np.tile(b2[None, :], (128, 1)).astype(np.float32)

    in_maps = []
    for c in range(N_CORES):
        xs = np.zeros((N_FEAT_IN, NPAD), np.float32)
        xs[:, :NPC] = x[c * NPC : (c + 1) * NPC].T
        dv = np.zeros(NPAD, np.float32)
        dv[:NPC] = dinv[c * NPC : (c + 1) * NPC]
        in_maps.append(
            {
                "xT": xs.astype(BF16),
                "w1": W1,
                "w2p": w2pad,
                "b1r": b1rep,
                "b2r": b2rep,
                "dinv_wr": np.ascontiguousarray(
                    dv.reshape(NWIN, 128).T
                ).astype(np.float32),
                "idxs": idx_dram[c],
                "ohd": oh_dram[c],
            }
        )

    return nc, in_maps


def kernel(x, edge_index, W1, b1, W2, b2, _trace=False):
    from concourse.bass_utils import run_bass_kernel_spmd

    nc, in_maps = prepare(x, edge_index, W1, b1, W2, b2)
    res = run_bass_kernel_spmd(
        nc, in_maps, core_ids=list(range(N_CORES)), trace=_trace
    )
    out = np.concatenate(
        [res.results[c]["z"][:NPC] for c in range(N_CORES)], axis=0
    ).astype(np.float32)
    if _trace:
        kernel.last_exec_time_ns = res.exec_time_ns
        kernel.last_results = res
    return out



# revision 24
# speedup vs baseline: 1.7702x; 1.7702x over previous
"""Two-layer GCN (GCNConv x2) on 8 Trainium2 NeuronCores.

HW phase profiling showed the edge-aggregation phases dominate (3.8ms of
4.4ms) and are almost entirely DMA: random 256B row gathers from the HBM
feature table plus one-hot matrix loads. v3 attacks both:
  - tables are staged into SBUF per block (one 6.55MB streaming DMA) and
    gathered SBUF->SBUF in transpose mode (no HBM random-read latency);
    each 128-slot chunk is PE-transposed back to slot-major via the
    identity-matmul trick, copied PSUM->SBUF on the idle ACT engine
  - one-hot scatter matrices are generated on-chip (iota + is_equal
    against a 2-byte dst index per slot) instead of 128B/slot DMA loads
  - self-loops are folded into the accumulator init (no gather slots)

Self-contained: only numpy/ml_dtypes/concourse imports; all shapes hardcoded.
"""

import numpy as np
import ml_dtypes

N_NODES = 100000
N_FEAT_IN = 256
N_FEAT_H = 128
N_FEAT_OUT = 64
N_EDGES = 1600000
N_CORES = 8
NPC = N_NODES // N_CORES  # 12500 nodes per core
NPAD = 12800  # padded rows per core shard (100 windows of 128)
NWIN = NPAD // 128  # 100
QROWS = NPAD // 4  # 3200 rows per collective chunk (25 windows)
NBLK = 4  # table blocks == collective chunks
BLKROWS = N_CORES * QROWS  # 25600 rows per gathered block (< 32767 int16)
RANKS = BLKROWS // 128  # 200 sbuf-table rows per partition
JMAX = 28  # max chunks per gather batch

BF16 = ml_dtypes.bfloat16
FP8 = ml_dtypes.float8_e4m3

# Experiment knobs (read at prepare/build time; part of the cache key).
OPTS = {
    "selfloop_local": True,  # fold self-loops into acc init; no slots
    "sort_src": True,        # sort slots by source row within each cell
    "nq": 4,                 # SWDGE queues; HBM gathers round-robin (sbuf gather needs nq=1)
    "oh_mode": "gen",        # gen | fp8_cast | bf16_hw
    "table_sbuf": False,     # stage gather tables in SBUF (slower; needs nq=1)
    "single_packet": False,
    "local_copy": True,      # copy AG output Shared->Local DRAM before gather
    "skip": None,            # diagnostic: "gather" | "oh" | "mm" | "gather_oh"
}

_CACHE = {}


# ---------------------------------------------------------------------------
# Host preprocessing
# ---------------------------------------------------------------------------

def _wrap16(idx: np.ndarray) -> np.ndarray:
    """dma_gather index layout: idx i -> partition i%16, col i//16,
    replicated to the 8 groups of 16 partitions. Returns [128, n//16]."""
    n = idx.shape[0]
    arr = idx.reshape(n // 16, 16).T
    return np.ascontiguousarray(np.tile(arr, (8, 1)).astype(np.int16))


def _preprocess(edge_index: np.ndarray, opts):
    src = np.concatenate([edge_index[0], np.arange(N_NODES, dtype=np.int64)])
    dst = np.concatenate([edge_index[1], np.arange(N_NODES, dtype=np.int64)])
    deg = np.bincount(dst, minlength=N_NODES)
    dinv = (1.0 / np.sqrt(deg.astype(np.float64))).astype(np.float32)

    if opts["selfloop_local"]:
        # self-loop term is applied on-device from the local table rows
        src = src[:N_EDGES]
        dst = dst[:N_EDGES]

    core = (dst // NPC).astype(np.int64)
    loc = dst - core * NPC
    win = loc >> 7
    dl = (loc & 127).astype(np.int64)
    # source row inside its block's table: block q holds, core-major, each
    # core's rows [q*QROWS, (q+1)*QROWS)
    sc = src // NPC
    sl = src - sc * NPC
    blk = sl // QROWS
    bloc = (sc * QROWS + (sl % QROWS)).astype(np.int64)

    ncell = NBLK * NWIN
    # cells ordered (dst-quarter, blk, win-within-quarter) so aggregation of
    # quarter q finishes early and epilogue/AG2 of q overlaps the agg tail
    WQ = NWIN // 4  # 25 windows per quarter
    cell_order = [
        (b, w)
        for q in range(4)
        for b in range(NBLK)
        for w in range(q * WQ, (q + 1) * WQ)
    ]
    cid_of = np.zeros((NBLK, NWIN), np.int64)
    for i, (b, w) in enumerate(cell_order):
        cid_of[b, w] = i
    cell = core * ncell + cid_of[blk, win]  # (dstcore, ordered cell)
    counts = np.bincount(cell, minlength=N_CORES * ncell).reshape(
        N_CORES, ncell
    )
    ccap = counts.max(axis=0)  # [ncell]
    chunks_c = (ccap + 127) // 128  # chunks per ordered cell; may be 0
    cap_flat = chunks_c * 128
    cell_off = np.concatenate([[0], np.cumsum(cap_flat)])
    tot = int(cell_off[-1])

    # slot position of each edge inside its core's array
    if opts["sort_src"]:
        order = np.lexsort((bloc, cell))
    else:
        order = np.argsort(cell, kind="stable")
    cell_s = cell[order]
    counts_flat = np.bincount(cell, minlength=N_CORES * ncell)
    run_starts = np.concatenate([[0], np.cumsum(counts_flat)])[:-1]
    rank = np.arange(len(cell_s)) - run_starts[cell_s]
    core_s = cell_s // ncell
    cid_s = cell_s % ncell
    pos = cell_off[cid_s] + rank

    if opts["table_sbuf"]:
        # SBUF table layout: row r -> partition r//RANKS, rank r%RANKS;
        # gather idx encoding = rank*128 + partition
        bloc = (bloc % RANKS) * 128 + bloc // RANKS
    src_arr = np.zeros((N_CORES, tot), np.int16)
    dst_arr = np.full((N_CORES, tot), -1, np.int64)
    src_arr[core_s, pos] = bloc[order].astype(np.int16)
    dst_arr[core_s, pos] = dl[order]

    # batches: per (quarter, block), consecutive cells <= JMAX chunks
    batches = []  # {blk, qtr, off(chunks), cells: [(win, nchunks)...], nch}
    k = 0
    for i, (b, w) in enumerate(cell_order):
        nc_w = int(chunks_c[i])
        q = w // WQ
        if nc_w == 0:
            continue
        cur = batches[-1] if batches else None
        if (
            cur is None or cur["blk"] != b or cur["qtr"] != q
            or cur["nch"] + nc_w > JMAX
        ):
            cur = {"blk": b, "qtr": q, "off": k, "cells": [], "nch": 0}
            batches.append(cur)
        cur["cells"].append((w, nc_w))
        cur["nch"] += nc_w
        k += nc_w
    ktot = k
    assert ktot * 128 == tot

    # per-core gather index dram [128, ktot*8]
    idx_dram = np.zeros((N_CORES, 128, ktot * 8), np.int16)
    for c in range(N_CORES):
        cols = []
        for bt in batches:
            nb = bt["nch"]
            s = bt["off"] * 128
            cols.append(_wrap16(src_arr[c, s : s + nb * 128]))
        idx_dram[c] = np.hstack(cols)

    if opts["oh_mode"] == "gen":
        # 2-byte dst index per slot, -1 for padding; bf16 exact for [-1,127]
        oh_dram = np.ascontiguousarray(
            dst_arr.reshape(N_CORES, ktot, 128).transpose(0, 2, 1)
        ).astype(BF16)
    else:
        # one-hot scatter matrices: oh[c, p, k*128 + d] = 1 iff slot
        # (k*128 + p) of core c has dst-local d. Padding slots stay zero.
        oh = np.zeros((N_CORES, 128, ktot * 128), np.uint8)
        slots = np.arange(tot)
        p_of = slots % 128
        k_of = slots // 128
        for c in range(N_CORES):
            valid = dst_arr[c] >= 0
            oh[c, p_of[valid], k_of[valid] * 128 + dst_arr[c][valid]] = 0x38
        if opts["oh_mode"] == "bf16_hw":
            ohw = np.zeros((N_CORES, 128, ktot * 128), np.uint16)
            ohw[oh != 0] = 0x3F80
            oh_dram = ohw.view(BF16)
        else:
            oh_dram = oh.view(FP8)

    meta = {"batches": batches, "ktot": ktot, "opts": dict(opts)}
    return meta, dinv, idx_dram, oh_dram


# ---------------------------------------------------------------------------
# Device kernel
# ---------------------------------------------------------------------------

def _build_nc(meta, nstages=7, reps=1):
    """nstages: 1=A, 2=+AG1, 3=+aggregation1, 4=+epilogue1, 5=+AG2, 6=+agg2,
    7=full. Partial builds write acc1 prefix to z."""
    import concourse.bacc as bacc
    import concourse.mybir as mybir
    import concourse.tile as tile
    from concourse.masks import make_identity

    ktot = meta["ktot"]
    batches = meta["batches"]
    opts = meta["opts"]
    nq = opts["nq"]
    oh_mode = opts["oh_mode"]
    selfloop_local = opts["selfloop_local"]
    table_sbuf = opts["table_sbuf"]
    local_copy = opts.get("local_copy", False)
    skip = opts.get("skip")

    nc = bacc.Bacc(
        "TRN2", target_bir_lowering=False, debug=False, num_devices=N_CORES,
        num_swdge_queues=nq,
    )
    f32, bf16, i16 = mybir.dt.float32, mybir.dt.bfloat16, mybir.dt.int16
    f8 = mybir.dt.float8e4
    mult = mybir.AluOpType.mult

    xT = nc.dram_tensor("xT", [N_FEAT_IN, NPAD], bf16, kind="ExternalInput")
    w1 = nc.dram_tensor("w1", [N_FEAT_IN, N_FEAT_H], f32, kind="ExternalInput")
    w2p = nc.dram_tensor("w2p", [N_FEAT_H, 128], f32, kind="ExternalInput")
    b1r = nc.dram_tensor("b1r", [128, N_FEAT_H], f32, kind="ExternalInput")
    b2r = nc.dram_tensor("b2r", [128, N_FEAT_OUT], f32, kind="ExternalInput")
    dinv_wr = nc.dram_tensor("dinv_wr", [128, NWIN], f32, kind="ExternalInput")
    idxs = nc.dram_tensor("idxs", [128, ktot * 8], i16, kind="ExternalInput")
    if oh_mode == "gen":
        ohd = nc.dram_tensor("ohd", [128, ktot], bf16, kind="ExternalInput")
    else:
        oh_dram_dt = bf16 if oh_mode == "bf16_hw" else f8
        ohd = nc.dram_tensor(
            "ohd", [128, ktot * 128], oh_dram_dt, kind="ExternalInput"
        )
    z = nc.dram_tensor("z", [NPAD, N_FEAT_OUT], f32, kind="ExternalOutput")

    blk_shape = [128, RANKS * 128] if table_sbuf else [BLKROWS, 128]

    with tile.TileContext(nc) as tc:
        with (
            tc.tile_pool(name="dram", bufs=1, space="DRAM") as dram,
            tc.tile_pool(name="persist", bufs=1) as pers,
            tc.tile_pool(name="work", bufs=4) as work,
            tc.tile_pool(name="tabpool", bufs=1) as tabpool,
            tc.tile_pool(name="gpool", bufs=6) as gpool,
            tc.tile_pool(name="gcpool", bufs=2) as gcpool,
            tc.tile_pool(name="ohpool", bufs=4) as ohpool,
            tc.tile_pool(name="stagea", bufs=6) as sta,
            tc.tile_pool(name="psum_mm", bufs=2, space="PSUM") as psmm,
            tc.tile_pool(name="psum_tr", bufs=2, space="PSUM") as pstr,
            tc.tile_pool(name="psum_gtr", bufs=2, space="PSUM") as pgtr,
            tc.tile_pool(name="psum_cell", bufs=2, space="PSUM") as pscell,
        ):
            g1_sh = [
                dram.tile([QROWS, N_FEAT_H], bf16, name=f"g1_sh{q}")
                for q in range(NBLK)
            ]
            g2_sh = [
                dram.tile([QROWS, 128], bf16, name=f"g2_sh{q}")
                for q in range(NBLK)
            ]
            g1_blk = [
                nc.dram_tensor(f"g1_blk{q}", blk_shape, bf16, addr_space="Shared")
                for q in range(NBLK)
            ]
            g2_blk = [
                nc.dram_tensor(f"g2_blk{q}", blk_shape, bf16, addr_space="Shared")
                for q in range(NBLK)
            ]
            g1_loc = g2_loc = None
            if local_copy:
                g1_loc = [
                    dram.tile(blk_shape, bf16, name=f"g1_loc{q}")
                    for q in range(NBLK)
                ]
                g2_loc = [
                    dram.tile(blk_shape, bf16, name=f"g2_loc{q}")
                    for q in range(NBLK)
                ]

            # ---- constants ----
            w1a = pers.tile([128, N_FEAT_H], bf16, tag="w1a")
            w1b = pers.tile([128, N_FEAT_H], bf16, tag="w1b")
            w2t = pers.tile([N_FEAT_H, 128], bf16, tag="w2t")
            b1t = pers.tile([128, N_FEAT_H], f32, tag="b1t")
            b2t = pers.tile([128, N_FEAT_OUT], f32, tag="b2t")
            dnv = pers.tile([128, NWIN], f32, tag="dnv")
            idn = pers.tile([128, 128], bf16, tag="idn")
            nc.gpsimd.dma_start(out=w1a[:], in_=w1[0:128, :])
            nc.gpsimd.dma_start(out=w1b[:], in_=w1[128:256, :])
            nc.gpsimd.dma_start(out=w2t[:], in_=w2p[:])
            nc.sync.dma_start(out=b1t[:], in_=b1r[:])
            nc.sync.dma_start(out=b2t[:], in_=b2r[:])
            nc.sync.dma_start(out=dnv[:], in_=dinv_wr[:])
            make_identity(nc, idn[:])
            it_all = pers.tile([128, ktot * 8], i16, tag="it_all")
            nc.sync.dma_start(out=it_all[:], in_=idxs[:])
            if oh_mode == "gen":
                iot = pers.tile([128, 128], bf16, tag="iot")
                nc.gpsimd.iota(
                    iot[:], pattern=[[1, 128]], base=0, channel_multiplier=0,
                    allow_small_or_imprecise_dtypes=True,
                )
                dl_all = pers.tile([128, ktot], bf16, tag="dl_all")
                nc.gpsimd.dma_start(out=dl_all[:], in_=ohd[:])

            for _rep in range(reps):
                acc1 = []
                for w in range(NWIN):
                    acc1_t = pers.tile([128, N_FEAT_H], f32, tag=f"acc1_{w}")
                    if not selfloop_local:
                        nc.vector.memset(acc1_t[:], 0.0)
                    acc1.append(acc1_t)
                acc2 = []
                for w in range(NWIN):
                    acc2_t = pers.tile([128, N_FEAT_OUT], f32, tag=f"acc2_{w}")
                    if not selfloop_local:
                        nc.vector.memset(acc2_t[:], 0.0)
                    acc2.append(acc2_t)

                # ---- stage A: g1 = dinv * (x @ W1), node-partition layout ----
                for w in range(NWIN):
                    r0 = sta.tile([128, 128], bf16, tag="rhs0")
                    r1 = sta.tile([128, 128], bf16, tag="rhs1")
                    nc.sync.dma_start(
                        out=r0[:], in_=xT[0:128, w * 128 : (w + 1) * 128]
                    )
                    nc.sync.dma_start(
                        out=r1[:], in_=xT[128:256, w * 128 : (w + 1) * 128]
                    )
                    ps = psmm.tile([128, N_FEAT_H], f32, space="PSUM", tag="mm")
                    nc.tensor.matmul(
                        out=ps[:], lhsT=r0[:], rhs=w1a[:], start=True, stop=False
                    )
                    nc.tensor.matmul(
                        out=ps[:], lhsT=r1[:], rhs=w1b[:], start=False, stop=True
                    )
                    g1w = sta.tile([128, N_FEAT_H], bf16, tag="g1w")
                    nc.vector.tensor_scalar(
                        out=g1w[:], in0=ps[:], scalar1=dnv[:, w : w + 1],
                        scalar2=None, op0=mult,
                    )
                    if selfloop_local:
                        # self-loop msg: acc1 holds dinv[src]*h; epilogue
                        # multiplies by dinv[dst] (same node) later
                        nc.vector.tensor_scalar(
                            out=acc1[w][:], in0=ps[:], scalar1=dnv[:, w : w + 1],
                            scalar2=None, op0=mult,
                        )
                    q, wq = w // 25, w % 25
                    nc.sync.dma_start(
                        out=g1_sh[q][wq * 128 : (wq + 1) * 128, :], in_=g1w[:]
                    )

                if nstages >= 2:
                    for q in range(NBLK):
                        nc.gpsimd.collective_compute(
                            "AllGather",
                            mybir.AluOpType.bypass,
                            ins=[g1_sh[q].opt()],
                            outs=[g1_blk[q][:]],
                            replica_groups=[list(range(N_CORES))],
                        )

                # ---- aggregation over edge chunk batches ----
                copied1 = set()
                copied2 = set()

                def aggregate(blk_tabs, acc, nfeat, loc_tabs=None,
                              qtr=None, copied=None):
                    tab = None
                    src_tabs = blk_tabs
                    if local_copy:
                        blk_tabs = loc_tabs
                    for bi, bt in enumerate(batches):
                        if qtr is not None and bt["qtr"] != qtr:
                            continue
                        b, off, nb = bt["blk"], bt["off"], bt["nch"]
                        if local_copy and b not in copied:
                            nc.sync.dma_start(
                                out=loc_tabs[b][:], in_=src_tabs[b][:]
                            )
                            copied.add(b)
                        it = it_all[:, off * 8 : (off + nb) * 8]
                        dlt = dl_all[:, off : off + nb] if oh_mode == "gen" else None
                        if table_sbuf and b != cur_blk:
                            tab = tabpool.tile(
                                [128, RANKS * 128], bf16, tag="tab"
                            )
                            nc.sync.dma_start(out=tab[:], in_=blk_tabs[b][:])
                            cur_blk = b
                        G = gpool.tile(
                            [128, 1, nb * 128] if table_sbuf
                            else [128, nb, 128],
                            bf16, tag="G",
                        )
                        if skip not in ("gather", "gather_oh"):
                            if table_sbuf:
                                nc.gpsimd.dma_gather(
                                    out_ap=G[:],
                                    in_ap=tab[:],
                                    idxs_ap=it,
                                    num_idxs=nb * 128,
                                    num_idxs_reg=nb * 128,
                                    elem_size=128,
                                    transpose=True,
                                    single_packet=opts["single_packet"],
                                    queue_num=0,
                                    sbuf_tokens_per_rank=128,
                                    sbuf_free_dim_per_rank=256,
                                )
                            else:
                                nc.gpsimd.dma_gather(
                                    out_ap=G[:],
                                    in_ap=blk_tabs[b][:],
                                    idxs_ap=it,
                                    num_idxs=nb * 128,
                                    num_idxs_reg=nb * 128,
                                    elem_size=128,
                                    single_packet=opts["single_packet"],
                                    queue_num=bi % nq,
                                )
                        OH_b = None
                        if oh_mode != "gen" and skip not in ("oh", "gather_oh"):
                            OH_b = ohpool.tile(
                                [128, nb, 128], bf16, tag="OHB"
                            )
                            if oh_mode == "fp8_cast":
                                nc.gpsimd.dma_start(
                                    out=OH_b[:],
                                    in_=ohd[:, off * 128 : (off + nb) * 128],
                                )
                            else:
                                nc.sync.dma_start(
                                    out=OH_b[:],
                                    in_=ohd[:, off * 128 : (off + nb) * 128],
                                )
                        if skip == "mm":
                            continue
                        if table_sbuf:
                            # transpose all chunks of the batch to slot-major
                            # first, so cell accumulation chains stay clean
                            GC = gcpool.tile([128, nb, 128], bf16, tag="GC")
                            for j in range(nb):
                                trp = pgtr.tile(
                                    [128, 128], bf16, space="PSUM", tag="gtr"
                                )
                                nc.tensor.transpose(
                                    out=trp[:],
                                    in_=G[:, 0, j * 128 : (j + 1) * 128],
                                    identity=idn[:],
                                )
                                nc.scalar.activation(
                                    out=GC[:, j, :], in_=trp[:],
                                    func=mybir.ActivationFunctionType.Copy,
                                )
                        if oh_mode == "gen":
                            OH_b = ohpool.tile([128, nb, 128], bf16, tag="OHB")
                            nc.vector.tensor_tensor(
                                out=OH_b[:],
                                in0=dlt.unsqueeze(2).broadcast_to(
                                    [128, nb, 128]
                                ),
                                in1=iot[:].unsqueeze(1).broadcast_to(
                                    [128, nb, 128]
                                ),
                                op=mybir.AluOpType.is_equal,
                            )
                        j = 0
                        for w, ncw in bt["cells"]:
                            ps = pscell.tile(
                                [128, nfeat], f32, space="PSUM", tag="cell"
                            )
                            for kk in range(ncw):
                                lhs = OH_b[:, j, :]
                                if table_sbuf:
                                    rhs = GC[:, j, :nfeat]
                                else:
                                    rhs = G[:, j, :nfeat]
                                nc.tensor.matmul(
                                    out=ps[:], lhsT=lhs, rhs=rhs,
                                    start=(kk == 0), stop=(kk == ncw - 1),
                                )
                                j += 1
                            sl = acc[w][:]
                            nc.vector.tensor_add(out=sl, in0=sl, in1=ps[:])

                def epilogue1(w):
                    aw = acc1[w][:]
                    t1 = sta.tile([128, N_FEAT_H], f32, tag="ep1")
                    nc.vector.tensor_scalar(
                        out=t1[:], in0=aw, scalar1=dnv[:, w : w + 1],
                        scalar2=None, op0=mult,
                    )
                    nc.vector.tensor_add(out=t1[:], in0=t1[:], in1=b1t[:])
                    t3 = sta.tile([128, N_FEAT_H], bf16, tag="ep3")
                    nc.scalar.activation(
                        out=t3[:], in_=t1[:],
                        func=mybir.ActivationFunctionType.Relu,
                    )
                    trp = pstr.tile([128, 128], bf16, space="PSUM", tag="tr")
                    nc.tensor.transpose(out=trp[:], in_=t3[:], identity=idn[:])
                    t3T = sta.tile([128, 128], bf16, tag="ep3T")
                    nc.vector.tensor_copy(out=t3T[:], in_=trp[:])
                    g2ps = psmm.tile([128, 128], f32, space="PSUM", tag="mm")
                    nc.tensor.matmul(
                        out=g2ps[:], lhsT=t3T[:], rhs=w2t[:], start=True,
                        stop=True,
                    )
                    g2w = sta.tile([128, 128], bf16, tag="g2w")
                    nc.vector.tensor_scalar(
                        out=g2w[:], in0=g2ps[:], scalar1=dnv[:, w : w + 1],
                        scalar2=None, op0=mult,
                    )
                    if selfloop_local:
                        nc.vector.tensor_scalar(
                            out=acc2[w][:], in0=g2ps[:, :N_FEAT_OUT],
                            scalar1=dnv[:, w : w + 1], scalar2=None, op0=mult,
                        )
                    q, wq = w // 25, w % 25
                    nc.sync.dma_start(
                        out=g2_sh[q][wq * 128 : (wq + 1) * 128, :], in_=g2w[:]
                    )

                for qtr in range(4):
                    if nstages >= 3:
                        aggregate(g1_blk, acc1, N_FEAT_H, g1_loc,
                                  qtr=qtr, copied=copied1)
                    if nstages >= 4:
                        for w in range(qtr * 25, (qtr + 1) * 25):
                            epilogue1(w)
                    if nstages >= 5:
                        nc.gpsimd.collective_compute(
                            "AllGather",
                            mybir.AluOpType.bypass,
                            ins=[g2_sh[qtr].opt()],
                            outs=[g2_blk[qtr][:]],
                            replica_groups=[list(range(N_CORES))],
                        )

                # ---- layer-2: agg + epilogue z = dinv*acc2 + b2, per qtr ----
                for qtr in range(4):
                    if nstages >= 6:
                        aggregate(g2_blk, acc2, N_FEAT_OUT, g2_loc,
                                  qtr=qtr, copied=copied2)
                    for w in range(
                        qtr * 25, (qtr + 1) * 25
                    ) if nstages >= 7 else ():
                        sl = acc2[w][:]
                        nc.vector.tensor_scalar(
                            out=sl, in0=sl, scalar1=dnv[:, w : w + 1],
                            scalar2=None, op0=mult,
                        )
                        nc.vector.tensor_add(out=sl, in0=sl, in1=b2t[:])
                        nc.sync.dma_start(
                            out=z[w * 128 : (w + 1) * 128, :], in_=sl
                        )
                if nstages < 7:
                    for w in range(NWIN):
                        nc.sync.dma_start(
                            out=z[w * 128 : (w + 1) * 128, :],
                            in_=acc1[w][:, :N_FEAT_OUT],
                        )
    nc.compile()
    return nc


# ---------------------------------------------------------------------------
# Entry point
# ---------------------------------------------------------------------------

def prepare(x, edge_index, W1, b1, W2, b2):
    """Preprocess + build + compile; returns (nc, in_maps)."""
    x = np.asarray(x)
    edge_index = np.asarray(edge_index)
    W1 = np.asarray(W1, dtype=np.float32)
    b1 = np.asarray(b1, dtype=np.float32)
    W2 = np.asarray(W2, dtype=np.float32)
    b2 = np.asarray(b2, dtype=np.float32)

    key = (hash(edge_index.tobytes()), tuple(sorted(
        (k, str(v)) for k, v in OPTS.items()
    )))
    if key not in _CACHE:
        meta, dinv, idx_dram, oh_dram = _preprocess(edge_index, OPTS)
        nc = _build_nc(meta)
        _CACHE[key] = (meta, dinv, idx_dram, oh_dram, nc)
    meta, dinv, idx_dram, oh_dram, nc = _CACHE[key]

    w2pad = np.zeros((N_FEAT_H, 128), np.float32)
    w2pad[:, :N_FEAT_OUT] = W2
    b1rep = np.tile(b1[None, :], (128, 1)).astype(np.float32)
    b2rep = np.tile(b2[None, :], (128, 1)).astype(np.float32)

    in_maps = []
    for c in range(N_CORES):
        xs = np.zeros((N_FEAT_IN, NPAD), np.float32)
        xs[:, :NPC] = x[c * NPC : (c + 1) * NPC].T
        dv = np.zeros(NPAD, np.float32)
        dv[:NPC] = dinv[c * NPC : (c + 1) * NPC]
        in_maps.append(
            {
                "xT": xs.astype(BF16),
                "w1": W1,
                "w2p": w2pad,
                "b1r": b1rep,
                "b2r": b2rep,
                "dinv_wr": np.ascontiguousarray(
                    dv.reshape(NWIN, 128).T
                ).astype(np.float32),
                "idxs": idx_dram[c],
                "ohd": oh_dram[c],
            }
        )

    return nc, in_maps


def kernel(x, edge_index, W1, b1, W2, b2, _trace=False):
    from concourse.bass_utils import run_bass_kernel_spmd

    nc, in_maps = prepare(x, edge_index, W1, b1, W2, b2)
    res = run_bass_kernel_spmd(
        nc, in_maps, core_ids=list(range(N_CORES)), trace=_trace
    )
    out = np.concatenate(
        [res.results[c]["z"][:NPC] for c in range(N_CORES)], axis=0
    ).astype(np.float32)
    if _trace:
        kernel.last_exec_time_ns = res.exec_time_ns
        kernel.last_results = res
    return out
